# revision 26
# baseline (speedup 1.0000x reference)
import numpy as np

EPS = 1e-5
NC = 8
H = W = 3
N = 9
OC = 32
CP = 7
HN = 256
P = 128  # partitions


def _fold_consts(inp):
    """Host-side folding of all weights into matmul-ready constants."""
    f = lambda k: np.asarray(inp[k], np.float32)
    # image branch
    s1 = f('g1') / np.sqrt(f('v1') + EPS)
    A1 = f('w1')[:, 0] * f('wv')[0, 0] * s1 if 'wv' in inp else None
    return None


def _build(inputs):
    import concourse.bass as bass
    import concourse.bacc as bacc
    import concourse.tile as tile
    from concourse import mybir
    from concourse.bass_utils import run_bass_kernel_spmd

    dt = mybir.dt
    AF = mybir.ActivationFunctionType
    ALU = mybir.AluOpType

    x = np.asarray(inputs['x'], np.float32)
    xp = np.asarray(inputs['x_param'], np.float32)
    B = x.shape[0]
    Bc = B // NC            # rows per core
    G = Bc // P             # 128 g-groups per partition

    g = lambda k: np.asarray(inputs[k], np.float32)

    # ---------------- host-side constant folding ----------------
    # image branch (CIN=1, IC=1)
    wq, wk, wv = g('wq'), g('wk'), g('wv')
    c0 = float(wq[0, 0] * wk[0, 0])           # energy scale for image branch
    s1 = g('g1') / np.sqrt(g('v1') + EPS)
    A1 = g('w1')[:, 0] * wv[0, 0] * s1        # [32]
    C1 = (g('b1') - g('m1')) * s1 + g('be1')
    s2 = g('g2') / np.sqrt(g('v2') + EPS)
    W2i = g('w2') * s2[:, None]               # [32,32] row-scaled
    C2i = (g('b2') - g('m2')) * s2 + g('be2')
    # param branch
    wqp, wkp, wvp = g('wqp'), g('wkp'), g('wvp')
    s1p = g('g1p') / np.sqrt(g('v1p') + EPS)
    W1v = (g('w1p') * s1p[:, None]) @ wvp     # [32,7]
    C1p = (g('b1p') - g('m1p')) * s1p + g('be1p')
    s2p = g('g2p') / np.sqrt(g('v2p') + EPS)
    W2p = g('w2p') * s2p[:, None]
    C2p = (g('b2p') - g('m2p')) * s2p + g('be2p')
    fw1, fb1, fw2, fb2 = g('fw1'), g('fb1'), g('fw2'), g('fb2')

    # y1cat feature order: j<32 param branch, j>=32 image branch
    # oc rows are ordered r = c*9 + i (c: 0-6 param chans, 7 img; i: position)
    def w1block(positions):
        # lhsT [72, len(positions)*64] for the given output positions
        m = np.zeros((72, len(positions) * 64), np.float32)
        for ii, i in enumerate(positions):
            for c in range(7):
                m[c * 9 + i, ii * 64:ii * 64 + 32] = W1v[:, c]
            m[7 * 9 + i, ii * 64 + 32:ii * 64 + 64] = A1
        return m
    b1all = np.concatenate([C1p, C1]).astype(np.float32)          # [64]
    b1pair = np.concatenate([b1all, b1all]).astype(np.float32)    # [128]

    W2s = np.zeros((64, 64), np.float32)      # lhsT[k=y1feat, m=y2feat]
    W2s[0:32, 0:32] = W2p.T
    W2s[32:64, 32:64] = W2i.T
    b2all = np.concatenate([C2p, C2i]).astype(np.float32)
    W2pair = np.zeros((128, 128), np.float32)
    W2pair[0:64, 0:64] = W2s
    W2pair[64:128, 64:128] = W2s
    b2pair = np.concatenate([b2all, b2all]).astype(np.float32)

    # fc1 weight rearranged per position i: rows = (i_local, j), cols = hn
    def catidx(j, i):
        if j < 32:
            return 288 + j * 9 + i            # param block of cat
        return (j - 32) * 9 + i               # image block of cat

    # fc1 weights as fp8 DoubleRow k-subtile pairs.
    # y2cat subtile S holds: S<4 -> rows (ii*64+j) = feature (j, i=2S+ii);
    # S=4 -> rows j<64 = feature (j, i=8), rows 64:128 zero; S=5 -> zero.
    def m1row(S, k):
        if S < 4:
            ii, j = k // 64, k % 64
            return catidx(j, 2 * S + ii)
        if S == 4 and k < 64:
            return catidx(k, 8)
        return None
    M1DR = np.zeros((2, 3, 128, 2, 128), np.float32)  # [half, pair, k, s, m]
    for h in range(2):
        for p3 in range(3):
            for s in range(2):
                S = 2 * p3 + s
                for k in range(128):
                    ci = m1row(S, k)
                    if ci is not None:
                        M1DR[h, p3, k, s, :] = fw1[h * 128:(h + 1) * 128, ci]

    fw2T = fw2.T.astype(np.float32)           # [256, 2]
    FW2DR = np.zeros((128, 2, 32), np.float32)  # M=32: dual-fp8 ldweights needs M>=32
    FW2DR[:, 0, 0:2] = fw2T[0:128]
    FW2DR[:, 1, 0:2] = fw2T[128:256]

    # ---- pack weight constants into one [128, Fw] tensor ----
    cols = {}
    off = 0
    def put(name, arr, row0=0):
        nonlocal off
        a = np.zeros((128, arr.shape[1]), np.float32)
        a[row0:row0 + arr.shape[0]] = arr
        cols[name] = (off, arr.shape[1], row0 + arr.shape[0], row0)
        off += arr.shape[1]
        return a
    blocks = []
    blocks.append(put('id', np.eye(128, dtype=np.float32)))
    blocks.append(put('w2pair', W2pair))
    for t in range(4):
        blocks.append(put(f'w1quad{t}', w1block([2 * t, 2 * t + 1])))
    blocks.append(put('w1s', w1block([8])))
    blocks.append(put('w2s', W2s))
    # q/k weights for the PE-side flipped matmul: lhsT = xpT slice [63, 128],
    # rhs = Wqk [63, 18]; Wqk[c*9+n, i] = wqp[c]*(n==i) (cols 0-8) / wkp (9-17)
    Wqk = np.zeros((63, 18), np.float32)
    for c in range(7):
        for i in range(9):
            Wqk[c * 9 + i, i] = wqp[0, c]
            Wqk[c * 9 + i, 9 + i] = wkp[0, c]
    blocks.append(put('wqk', Wqk))
    cw_np = np.concatenate(blocks, axis=1)
    import ml_dtypes
    cw_np_bf = cw_np.astype(ml_dtypes.bfloat16)
    Fw = cw_np.shape[1]

    # ---- fp8 weight constants (fc1 DR pairs + fc2 DR) ----
    cols8 = {}
    off8 = 0
    blocks8 = []
    def put8(name, arr):  # arr [128, w]
        nonlocal off8
        cols8[name] = (off8, arr.shape[1])
        off8 += arr.shape[1]
        blocks8.append(arr.astype(np.float32))
    for h in range(2):
        for p3 in range(3):
            put8(f'fc1_{h}_{p3}', M1DR[h, p3].reshape(128, 256))
    put8('fw2dr', FW2DR.reshape(128, 64))
    cw8_np = np.concatenate(blocks8, axis=1).astype(ml_dtypes.float8_e4m3)
    F8 = cw8_np.shape[1]

    cb_np = np.zeros((128, 8), np.float32)
    cb_np[:, 0] = b1pair
    cb_np[:, 1] = b2pair
    cb_np[:, 2] = fb1[0:128]
    cb_np[:, 3] = fb1[128:256]
    cb_np[0:2, 4] = fb2 * 0.5
    cb_np[0:64, 5] = b1all
    cb_np[0:64, 6] = b2all

    # ---------------- build the bass program ----------------
    nc = bacc.Bacc("TRN2", target_bir_lowering=False, debug=False)
    f32, f32r, bf16 = dt.float32, dt.float32r, dt.bfloat16
    fp8 = dt.float8e4
    DR = mybir.MatmulPerfMode.DoubleRow

    x_d = nc.dram_tensor("xin", [Bc * 9], f32, kind="ExternalInput").ap()
    xp_d = nc.dram_tensor("xpin", [Bc * 63], f32, kind="ExternalInput").ap()
    xpt_d = nc.dram_tensor("xptin", [63, Bc], bf16, kind="ExternalInput").ap()
    cw_d = nc.dram_tensor("cw", [128, Fw], bf16, kind="ExternalInput").ap()
    cw8_d = nc.dram_tensor("cw8", [128, F8], fp8, kind="ExternalInput").ap()
    cb_d = nc.dram_tensor("cb", [128, 8], f32, kind="ExternalInput").ap()
    y_d = nc.dram_tensor("yout", [2, P, G], f32, kind="ExternalOutput").ap()

    xv = x_d.rearrange("(p f) -> p f", p=P)     # [128, G*9]
    xpv = xp_d.rearrange("(p f) -> p f", p=P)   # [128, G*63]
    yv = y_d                                     # [2, 128, 128]

    NCH = 4                  # dma chunks over g
    GC = G // NCH            # 32 g per chunk
    NBLK = 4                 # blocks per chunk (8 g each)
    GB = GC // NBLK          # 8 g per block
    NGRP = 2                 # groups per block (4 g each)
    GG = GB // NGRP          # 4

    wqp_l = [float(v) for v in wqp[0]]
    wkp_l = [float(v) for v in wkp[0]]

    with tile.TileContext(nc) as tc:
        with (
            tc.tile_pool(name="consts", bufs=1) as pc,
            tc.tile_pool(name="pin", bufs=2) as pin,
            tc.tile_pool(name="pq", bufs=2) as pq,
            tc.tile_pool(name="patt", bufs=2) as pa,
            tc.tile_pool(name="pmm", bufs=2) as pm,
            tc.tile_pool(name="py2", bufs=2) as py2,
            tc.tile_pool(name="pys", bufs=2) as pys,
            tc.tile_pool(name="pps", bufs=2, space="PSUM") as pps,
        ):
            cw_t = pc.tile([128, Fw], bf16)
            nc.sync.dma_start(cw_t[:], cw_d)
            cw8_t = pc.tile([128, F8], fp8)
            nc.sync.dma_start(cw8_t[:], cw8_d)
            cb_t = pc.tile([128, 8], f32)
            nc.sync.dma_start(cb_t[:], cb_d)

            def wslice(name):
                o, w_, r1, r0 = cols[name]
                return cw_t[r0:r1, o:o + w_]

            def w8pair(name):
                o, w_ = cols8[name]
                return cw8_t[:, o:o + w_].rearrange("p (s m) -> p s m", s=2)

            ident = wslice('id')
            gidx = [0]

            for k in range(NCH):
                # ---- input DMA for this chunk ----
                xpc = pin.tile([128, GC * 63 + 16], bf16, tag="xp")
                nc.gpsimd.memset(xpc[:, GC * 63:], 0.0)
                nc.gpsimd.dma_start(xpc[:, 0:GC * 63], xpv[:, k * GC * 63:(k + 1) * GC * 63])
                xc = pin.tile([128, GC * 9 + 16], bf16, tag="x")
                nc.gpsimd.memset(xc[:, GC * 9:], 0.0)
                nc.gpsimd.dma_start(xc[:, 0:GC * 9],
                                    xv[:, k * GC * 9:(k + 1) * GC * 9])
                xptc = pin.tile([64, GC * 128], bf16, tag="xpt")
                nc.sync.dma_start(
                    xptc[0:63, :], xpt_d[:, k * GC * 128:(k + 1) * GC * 128])

                for j in range(NBLK):
                    gb0 = j * GB  # g offset within chunk
                    # views for this block (8 g)
                    xpb = xpc[:, gb0 * 63:(gb0 + GB) * 63].rearrange(
                        "p (g c n) -> p g c n", c=7, n=9)
                    xb = xc[:, gb0 * 9:(gb0 + GB) * 9].rearrange(
                        "p (g n) -> p g n", n=9)

                    # --- q/k via PE (flipped: lhsT = xpT data, rhs = Wqk) ---
                    psqk = pps.tile([128, 512], f32, tag="h")
                    for g in range(GB):
                        gg0 = (gb0 + g) * 128
                        nc.tensor.matmul(
                            psqk[:, g * 18:(g + 1) * 18],
                            xptc[0:63, gg0:gg0 + 128],
                            wslice('wqk'), start=True, stop=True)
                    Q = pq.tile([128, GB * 18], bf16, tag="Q")
                    nc.vector.tensor_copy(Q[:], psqk[:, 0:GB * 18])
                    qb = Q.rearrange("p (g t) -> p g t", t=18)

                    # --- energies (no pad; exp writes into T2/TI slot layouts) ---
                    E = pa.tile([128, GB * 81], bf16, tag="E")
                    E4 = E.rearrange("p (g i n) -> p g i n", i=9, n=9)
                    qpA = qb[:, :, 0:9].unsqueeze(3).broadcast_to((128, GB, 9, 9))
                    kpA = qb[:, :, 9:18].unsqueeze(2).broadcast_to((128, GB, 9, 9))
                    nc.gpsimd.tensor_mul(E4[:], qpA, kpA)
                    EI = pa.tile([128, GB * 81], bf16, tag="EI")
                    EI4 = EI.rearrange("p (g i n) -> p g i n", i=9, n=9)
                    xiA = xb.unsqueeze(3).broadcast_to((128, GB, 9, 9))
                    xnA = xb.unsqueeze(2).broadcast_to((128, GB, 9, 9))
                    nc.gpsimd.tensor_mul(EI4[:], xiA, xnA)

                    # --- T2: slots 0-6 = xp_c * E2, slot 7 = E2 (for D) ---
                    T2 = pa.tile([128, GB * 720], bf16, tag="T2")
                    T25 = T2.rearrange("p (g c i n) -> p g c i n", c=8, i=9, n=10)
                    nc.gpsimd.memset(T25[:, :, 7, :, 9], 0.0)
                    nc.scalar.activation(T25[:, :, 7, :, 0:9], E4[:], AF.Exp)
                    E24 = T25[:, :, 7, :, :]  # [p, g, 9i, 10n]
                    xpb_u = xpb.unsqueeze(3)
                    ap10 = [list(p) for p in xpb_u.ap]
                    ap10[-1][1] = 10  # read 10 consecutive (1 slack elem, x0)
                    xpbA = bass.AP(xpb_u.tensor, xpb_u.offset, ap10).broadcast_to(
                        (128, GB, 7, 9, 10))
                    e2A = E24.unsqueeze(2).broadcast_to((128, GB, 7, 9, 10))
                    nc.vector.tensor_mul(T25[:, :, 0:7, :, :], xpbA, e2A)

                    # --- TI: slot 0 = x_n * EI2, slot 1 = EI2 (for D_img) ---
                    TI = pa.tile([128, GB * 180], bf16, tag="TI")
                    TI5 = TI.rearrange("p (g c i n) -> p g c i n", c=2, i=9, n=10)
                    nc.gpsimd.memset(TI5[:, :, 1, :, 9], 0.0)
                    nc.scalar.activation(TI5[:, :, 1, :, 0:9], EI4[:], AF.Exp,
                                         scale=c0)
                    xb_u = xb.unsqueeze(2)
                    xap10 = [list(p) for p in xb_u.ap]
                    xap10[-1][1] = 10
                    xbA = bass.AP(xb_u.tensor, xb_u.offset, xap10).broadcast_to(
                        (128, GB, 9, 10))
                    nc.gpsimd.tensor_mul(TI5[:, :, 0, :, :], xbA,
                                         TI5[:, :, 1, :, :])

                    # --- tree-add reductions over n (replaces TensorReduce) ---
                    L1 = pa.tile([128, GB * 360], bf16, tag="L1")
                    L1v = L1.rearrange("p (g c i n) -> p g c i n", c=8, i=9, n=5)
                    nc.vector.tensor_tensor(
                        L1v[:], T25[:, :, :, :, 0:5], T25[:, :, :, :, 5:10],
                        op=ALU.add)
                    L2 = pa.tile([128, GB * 144], bf16, tag="L2")
                    L2v = L2.rearrange("p (g c i n) -> p g c i n", c=8, i=9, n=2)
                    nc.vector.tensor_tensor(
                        L2v[:], L1v[:, :, :, :, 0:2], L1v[:, :, :, :, 2:4],
                        op=ALU.add)
                    GT = pa.tile([128, GB * 72], f32, tag="GT")
                    GTv = GT.rearrange("p (g c i) -> p g c i", c=8, i=9)
                    nc.vector.tensor_tensor(
                        GTv[:], L2v[:, :, :, :, 0], L2v[:, :, :, :, 1], op=ALU.add)
                    nc.vector.tensor_tensor(
                        GTv[:], GTv[:], L1v[:, :, :, :, 4], op=ALU.add)

                    M1i = pa.tile([128, GB * 90], bf16, tag="M1i")
                    M1v = M1i.rearrange("p (g c i n) -> p g c i n", c=2, i=9, n=5)
                    nc.gpsimd.tensor_tensor(
                        M1v[:], TI5[:, :, :, :, 0:5], TI5[:, :, :, :, 5:10],
                        op=ALU.add)
                    M2 = pa.tile([128, GB * 36], bf16, tag="M2i")
                    M2v = M2.rearrange("p (g c i n) -> p g c i n", c=2, i=9, n=2)
                    nc.gpsimd.tensor_tensor(
                        M2v[:], M1v[:, :, :, :, 0:2], M1v[:, :, :, :, 2:4],
                        op=ALU.add)
                    GI2 = pa.tile([128, GB * 18], f32, tag="GI2")
                    GIv = GI2.rearrange("p (g c i) -> p g c i", c=2, i=9)
                    nc.gpsimd.tensor_tensor(
                        GIv[:], M2v[:, :, :, :, 0], M2v[:, :, :, :, 1], op=ALU.add)
                    nc.gpsimd.tensor_tensor(
                        GIv[:], GIv[:], M1v[:, :, :, :, 4], op=ALU.add)

                    # --- reciprocals of the two denominators ---
                    R = pa.tile([128, GB * 18], f32, tag="R")
                    Rv = R.rearrange("p (g t) -> p g t", t=18)
                    nc.vector.reciprocal(Rv[:, :, 0:9], GTv[:, :, 7, :])
                    nc.vector.reciprocal(Rv[:, :, 9:18], GIv[:, :, 1, :])

                    # --- attention outputs, laid out (g, c*9+i) for transpose ---
                    OCt = pa.tile([128, GB * 72], bf16, tag="OC")
                    OCv = OCt.rearrange("p (g c i) -> p g c i", c=8, i=9)
                    rpA = Rv[:, :, 0:9].unsqueeze(2).broadcast_to((128, GB, 7, 9))
                    nc.gpsimd.tensor_mul(OCv[:, :, 0:7, :], GTv[:, :, 0:7, :], rpA)
                    nc.gpsimd.tensor_mul(OCv[:, :, 7, :], GIv[:, :, 0, :],
                                         Rv[:, :, 9:18])

                    # ---- PE stage: 2 groups of 4 g ----
                    for h2 in range(NGRP):
                        ggl = [h2 * GG + t for t in range(GG)]
                        ps_tr = pps.tile([128, 512], bf16, tag="trans")
                        for t, gg in enumerate(ggl):
                            nc.tensor.transpose(
                                ps_tr[0:72, t * 128:(t + 1) * 128],
                                OCt[:, gg * 72:(gg + 1) * 72],
                                ident)
                        oct = pm.tile([128, 512], bf16, tag="oct")
                        nc.vector.tensor_copy(oct[0:72, :], ps_tr[0:72, :])
                        octr = oct

                        y2cat = py2.tile([128, 3072], fp8, tag="y2cat")
                        y2v = y2cat.rearrange("p (s n) -> p s n", s=6)
                        if gidx[0] < 2:
                            # zero the never-written fp8 slots once per buffer
                            nc.gpsimd.memset(y2cat[64:128, 2048:2560], 0.0)
                            nc.gpsimd.memset(y2cat[:, 2560:3072], 0.0)
                        gidx[0] += 1

                        def evac(eng, dst, src, bias, rows=128):
                            # psum f32 -> sbuf relu(x + bias), engine-selectable
                            if eng == 'A':
                                nc.scalar.activation(dst, src, AF.Relu, bias=bias)
                            elif eng == 'P':
                                nc.gpsimd.tensor_scalar(
                                    dst, src, bias, 0.0, op0=ALU.add, op1=ALU.max)
                            else:
                                nc.vector.tensor_scalar(
                                    dst, src, bias, 0.0, op0=ALU.add, op1=ALU.max)

                        Y1E = ['A', 'D', 'A', 'A', 'D']
                        Y2E = ['A', 'D', 'A', 'D', 'A']
                        for t in range(5):
                            ps1 = pps.tile([128, 512], f32, tag="y1")
                            if t < 4:
                                nc.tensor.matmul(
                                    ps1[:], wslice(f'w1quad{t}'),
                                    octr[0:72, :],
                                    start=True, stop=True)
                                y1sb = pm.tile([128, 512], bf16, tag="y1sb")
                                evac(Y1E[t], y1sb[:], ps1[:], cb_t[:, 0:1])
                                ps2 = pps.tile([128, 512], f32, tag="y2")
                                nc.tensor.matmul(
                                    ps2[:], wslice('w2pair'),
                                    y1sb[:],
                                    start=True, stop=True)
                                evac(Y2E[t], y2cat[:, t * 512:(t + 1) * 512],
                                     ps2[:], cb_t[:, 1:2])
                            else:
                                nc.tensor.matmul(
                                    ps1[0:64, :], wslice('w1s'),
                                    octr[0:72, :], start=True, stop=True)
                                y1sb = pm.tile([128, 512], bf16, tag="y1sb")
                                evac(Y1E[t], y1sb[0:64, :], ps1[0:64, :],
                                     cb_t[0:64, 5:6])
                                ps2 = pps.tile([128, 512], f32, tag="y2")
                                nc.tensor.matmul(
                                    ps2[0:64, :], wslice('w2s'),
                                    y1sb[0:64, :],
                                    start=True, stop=True)
                                evac(Y2E[t], y2cat[0:64, 2048:2560],
                                     ps2[0:64, :], cb_t[0:64, 6:7])

                        h8 = pm.tile([128, 1024], fp8, tag="h8")
                        h8v = h8.rearrange("p (s n) -> p s n", s=2)
                        for ch in range(2):
                            psh = pps.tile([128, 512], f32, tag="h")
                            for p3 in range(3):
                                nc.tensor.matmul(
                                    psh[:], w8pair(f'fc1_{ch}_{p3}'),
                                    y2v[:, 2 * p3:2 * p3 + 2, :],
                                    start=(p3 == 0), stop=(p3 == 2),
                                    perf_mode=DR)
                            evac('A' if ch == 0 else 'D',
                                 h8[:, ch * 512:(ch + 1) * 512], psh[:],
                                 cb_t[:, 2 + ch:3 + ch])

                        psy32 = pps.tile([32, 512], f32, tag="y2")
                        nc.tensor.matmul(psy32[:], w8pair('fw2dr'), h8v,
                                         start=True, stop=True, perf_mode=DR)
                        psy = psy32[0:2, :]

                        # sigmoid(x) = 0.5*tanh(0.5*x) + 0.5  (keeps exp table set)
                        if j == 0 and h2 == 0:
                            Ysb = pys.tile([2, GC * 128], f32, tag="Y")
                        gl0 = j * GB + h2 * GG  # g offset within chunk
                        yview = Ysb.rearrange("c (b g) -> c g b", g=GC)
                        src = psy.rearrange("c (g b) -> c g b", g=GG)
                        nc.scalar.activation(
                            yview[:, gl0:gl0 + GG, :], src, AF.Tanh, scale=0.5,
                            bias=cb_t[0:2, 4:5])

                # ---- output DMA for the chunk ----
                ydst = yv[:, :, k * GC:(k + 1) * GC]  # [2, 128, GC]
                ysrc = Ysb.rearrange("c (b g) -> c b g", g=GC)
                nc.sync.dma_start(ydst, ysrc)

    nc.compile()

    in_maps = []
    for core in range(NC):
        sl = slice(core * Bc, (core + 1) * Bc)
        xpt = np.ascontiguousarray(
            xp[sl].reshape(128, Bc // 128, 63).transpose(2, 1, 0)
        ).astype(ml_dtypes.bfloat16)
        in_maps.append({
            "xin": np.ascontiguousarray(x[sl].reshape(-1)),
            "xpin": np.ascontiguousarray(xp[sl].reshape(-1)),
            "xptin": xpt.reshape(63, Bc),
            "cw": cw_np_bf,
            "cw8": cw8_np,
            "cb": cb_np,
        })
    return nc, in_maps


def kernel(**inputs):
    from concourse.bass_utils import run_bass_kernel_spmd
    nc, in_maps = _build(inputs)
    kernel._last_nc = nc
    res = run_bass_kernel_spmd(nc, in_maps, core_ids=list(range(NC)))
    kernel._last_result = res
    Bc = in_maps[0]["xin"].size // 9
    outs = []
    for core in range(NC):
        yc = res.results[core]["yout"]          # [2, 128, G] = (cls, p, g)
        outs.append(yc.transpose(1, 2, 0).reshape(-1) * 0.5 + 0.5)  # rows r=p*G+g
    return np.concatenate(outs)



# revision 28
# speedup vs baseline: 1.5217x; 1.5217x over previous
import numpy as np

EPS = 1e-5
NC = 8
H = W = 3
N = 9
OC = 32
CP = 7
HN = 256
P = 128  # partitions


def _fold_consts(inp):
    """Host-side folding of all weights into matmul-ready constants."""
    f = lambda k: np.asarray(inp[k], np.float32)
    # image branch
    s1 = f('g1') / np.sqrt(f('v1') + EPS)
    A1 = f('w1')[:, 0] * f('wv')[0, 0] * s1 if 'wv' in inp else None
    return None


def _build(inputs):
    import concourse.bass as bass
    import concourse.bacc as bacc
    import concourse.tile as tile
    from concourse import mybir
    from concourse.bass_utils import run_bass_kernel_spmd

    dt = mybir.dt
    AF = mybir.ActivationFunctionType
    ALU = mybir.AluOpType

    x = np.asarray(inputs['x'], np.float32)
    xp = np.asarray(inputs['x_param'], np.float32)
    B = x.shape[0]
    Bc = B // NC            # rows per core
    G = Bc // P             # 128 g-groups per partition

    g = lambda k: np.asarray(inputs[k], np.float32)

    # ---------------- host-side constant folding ----------------
    # image branch (CIN=1, IC=1)
    wq, wk, wv = g('wq'), g('wk'), g('wv')
    c0 = float(wq[0, 0] * wk[0, 0])           # energy scale for image branch
    s1 = g('g1') / np.sqrt(g('v1') + EPS)
    A1 = g('w1')[:, 0] * wv[0, 0] * s1        # [32]
    C1 = (g('b1') - g('m1')) * s1 + g('be1')
    s2 = g('g2') / np.sqrt(g('v2') + EPS)
    W2i = g('w2') * s2[:, None]               # [32,32] row-scaled
    C2i = (g('b2') - g('m2')) * s2 + g('be2')
    # param branch
    wqp, wkp, wvp = g('wqp'), g('wkp'), g('wvp')
    s1p = g('g1p') / np.sqrt(g('v1p') + EPS)
    W1v = (g('w1p') * s1p[:, None]) @ wvp     # [32,7]
    C1p = (g('b1p') - g('m1p')) * s1p + g('be1p')
    s2p = g('g2p') / np.sqrt(g('v2p') + EPS)
    W2p = g('w2p') * s2p[:, None]
    C2p = (g('b2p') - g('m2p')) * s2p + g('be2p')
    fw1, fb1, fw2, fb2 = g('fw1'), g('fb1'), g('fw2'), g('fb2')

    # y1cat feature order: j<32 param branch, j>=32 image branch
    # oc rows are ordered r = c*9 + i (c: 0-6 param chans, 7 img; i: position)
    def w1block(positions):
        # lhsT [72, len(positions)*64] for the given output positions
        m = np.zeros((72, len(positions) * 64), np.float32)
        for ii, i in enumerate(positions):
            for c in range(7):
                m[c * 9 + i, ii * 64:ii * 64 + 32] = W1v[:, c]
            m[7 * 9 + i, ii * 64 + 32:ii * 64 + 64] = A1
        return m
    b1all = np.concatenate([C1p, C1]).astype(np.float32)          # [64]
    b1pair = np.concatenate([b1all, b1all]).astype(np.float32)    # [128]

    W2s = np.zeros((64, 64), np.float32)      # lhsT[k=y1feat, m=y2feat]
    W2s[0:32, 0:32] = W2p.T
    W2s[32:64, 32:64] = W2i.T
    b2all = np.concatenate([C2p, C2i]).astype(np.float32)
    W2pair = np.zeros((128, 128), np.float32)
    W2pair[0:64, 0:64] = W2s
    W2pair[64:128, 64:128] = W2s
    b2pair = np.concatenate([b2all, b2all]).astype(np.float32)

    # fc1 weight rearranged per position i: rows = (i_local, j), cols = hn
    def catidx(j, i):
        if j < 32:
            return 288 + j * 9 + i            # param block of cat
        return (j - 32) * 9 + i               # image block of cat

    # fc1 weights as fp8 DoubleRow k-subtile pairs.
    # y2cat subtile S holds: S<4 -> rows (ii*64+j) = feature (j, i=2S+ii);
    # S=4 -> rows j<64 = feature (j, i=8), rows 64:128 zero; S=5 -> zero.
    def m1row(S, k):
        if S < 4:
            ii, j = k // 64, k % 64
            return catidx(j, 2 * S + ii)
        if S == 4 and k < 64:
            return catidx(k, 8)
        return None
    M1DR = np.zeros((2, 3, 128, 2, 128), np.float32)  # [half, pair, k, s, m]
    for h in range(2):
        for p3 in range(3):
            for s in range(2):
                S = 2 * p3 + s
                for k in range(128):
                    ci = m1row(S, k)
                    if ci is not None:
                        M1DR[h, p3, k, s, :] = fw1[h * 128:(h + 1) * 128, ci]

    fw2T = fw2.T.astype(np.float32)           # [256, 2]
    FW2DR = np.zeros((128, 2, 32), np.float32)  # M=32: dual-fp8 ldweights needs M>=32
    FW2DR[:, 0, 0:2] = fw2T[0:128]
    FW2DR[:, 1, 0:2] = fw2T[128:256]

    # ---- pack weight constants into one [128, Fw] tensor ----
    cols = {}
    off = 0
    def put(name, arr, row0=0):
        nonlocal off
        a = np.zeros((128, arr.shape[1]), np.float32)
        a[row0:row0 + arr.shape[0]] = arr
        cols[name] = (off, arr.shape[1], row0 + arr.shape[0], row0)
        off += arr.shape[1]
        return a
    blocks = []
    blocks.append(put('id', np.eye(128, dtype=np.float32)))
    blocks.append(put('w2pair', W2pair))
    for t in range(4):
        blocks.append(put(f'w1quad{t}', w1block([2 * t, 2 * t + 1])))
    blocks.append(put('w1s', w1block([8])))
    blocks.append(put('w2s', W2s))
    # q/k weights for the PE-side flipped matmul: lhsT = xpT slice [63, 128],
    # rhs = Wqk [63, 18]; Wqk[c*9+n, i] = wqp[c]*(n==i) (cols 0-8) / wkp (9-17)
    Wqk = np.zeros((63, 18), np.float32)
    for c in range(7):
        for i in range(9):
            Wqk[c * 9 + i, i] = wqp[0, c]
            Wqk[c * 9 + i, 9 + i] = wkp[0, c]
    blocks.append(put('wqk', Wqk))
    cw_np = np.concatenate(blocks, axis=1)
    import ml_dtypes
    cw_np_bf = cw_np.astype(ml_dtypes.bfloat16)
    Fw = cw_np.shape[1]

    # ---- fp8 weight constants (fc1 DR pairs + fc2 DR) ----
    cols8 = {}
    off8 = 0
    blocks8 = []
    def put8(name, arr):  # arr [128, w]
        nonlocal off8
        cols8[name] = (off8, arr.shape[1])
        off8 += arr.shape[1]
        blocks8.append(arr.astype(np.float32))
    for h in range(2):
        for p3 in range(3):
            put8(f'fc1_{h}_{p3}', M1DR[h, p3].reshape(128, 256))
    put8('fw2dr', FW2DR.reshape(128, 64))
    cw8_np = np.concatenate(blocks8, axis=1).astype(ml_dtypes.float8_e4m3)
    F8 = cw8_np.shape[1]

    cb_np = np.zeros((128, 8), np.float32)
    cb_np[:, 0] = b1pair
    cb_np[:, 1] = b2pair
    cb_np[:, 2] = fb1[0:128]
    cb_np[:, 3] = fb1[128:256]
    cb_np[0:2, 4] = fb2 * 0.5
    cb_np[0:64, 5] = b1all
    cb_np[0:64, 6] = b2all

    # ---------------- build the bass program ----------------
    nc = bacc.Bacc("TRN2", target_bir_lowering=False, debug=False)
    f32, f32r, bf16 = dt.float32, dt.float32r, dt.bfloat16
    fp8 = dt.float8e4
    DR = mybir.MatmulPerfMode.DoubleRow

    x_d = nc.dram_tensor("xin", [Bc * 9], f32, kind="ExternalInput").ap()
    xp_d = nc.dram_tensor("xpin", [Bc * 63], f32, kind="ExternalInput").ap()
    xpt_d = nc.dram_tensor("xptin", [63, Bc], bf16, kind="ExternalInput").ap()
    cw_d = nc.dram_tensor("cw", [128, Fw], bf16, kind="ExternalInput").ap()
    cw8_d = nc.dram_tensor("cw8", [128, F8], fp8, kind="ExternalInput").ap()
    cb_d = nc.dram_tensor("cb", [128, 8], f32, kind="ExternalInput").ap()
    y_d = nc.dram_tensor("yout", [2, P, G], f32, kind="ExternalOutput").ap()

    xv = x_d.rearrange("(p f) -> p f", p=P)     # [128, G*9]
    xpv = xp_d.rearrange("(p f) -> p f", p=P)   # [128, G*63]
    yv = y_d                                     # [2, 128, 128]

    NCH = 4                  # dma chunks over g
    GC = G // NCH            # 32 g per chunk
    NBLK = 4                 # blocks per chunk (8 g each)
    GB = GC // NBLK          # 8 g per block
    NGRP = 2                 # groups per block (4 g each)
    GG = GB // NGRP          # 4

    wqp_l = [float(v) for v in wqp[0]]
    wkp_l = [float(v) for v in wkp[0]]

    with tile.TileContext(nc) as tc:
        with (
            tc.tile_pool(name="consts", bufs=1) as pc,
            tc.tile_pool(name="pin", bufs=2) as pin,
            tc.tile_pool(name="pq", bufs=2) as pq,
            tc.tile_pool(name="patt", bufs=3) as pa,
            tc.tile_pool(name="pmm", bufs=2) as pm,
            tc.tile_pool(name="py2", bufs=2) as py2,
            tc.tile_pool(name="pys", bufs=2) as pys,
            tc.tile_pool(name="pps", bufs=2, space="PSUM") as pps,
        ):
            cw_t = pc.tile([128, Fw], bf16)
            nc.sync.dma_start(cw_t[:], cw_d)
            cw8_t = pc.tile([128, F8], fp8)
            nc.sync.dma_start(cw8_t[:], cw8_d)
            cb_t = pc.tile([128, 8], f32)
            nc.sync.dma_start(cb_t[:], cb_d)

            def wslice(name):
                o, w_, r1, r0 = cols[name]
                return cw_t[r0:r1, o:o + w_]

            def w8pair(name):
                o, w_ = cols8[name]
                return cw8_t[:, o:o + w_].rearrange("p (s m) -> p s m", s=2)

            ident = wslice('id')
            gidx = [0]

            for k in range(NCH):
                # ---- input DMA for this chunk ----
                xpc = pin.tile([128, GC * 63 + 16], bf16, tag="xp")
                nc.gpsimd.memset(xpc[:, GC * 63:], 0.0)
                nc.gpsimd.dma_start(xpc[:, 0:GC * 63], xpv[:, k * GC * 63:(k + 1) * GC * 63])
                xc = pin.tile([128, GC * 9 + 16], bf16, tag="x")
                nc.gpsimd.memset(xc[:, GC * 9:], 0.0)
                nc.gpsimd.dma_start(xc[:, 0:GC * 9],
                                    xv[:, k * GC * 9:(k + 1) * GC * 9])
                xptc = pin.tile([64, GC * 128], bf16, tag="xpt")
                nc.sync.dma_start(
                    xptc[0:63, :], xpt_d[:, k * GC * 128:(k + 1) * GC * 128])

                # ---- qp / kp for the whole chunk (32 g) ----
                Qc = pq.tile([128, GC * 18], bf16, tag="Q")
                qcv = Qc.rearrange("p (g t) -> p g t", t=18)
                xp3 = xpc[:, 0:GC * 63].rearrange("p (g c n) -> p g c n", c=7, n=9)
                for c in range(7):
                    if c == 0:
                        nc.vector.tensor_scalar_mul(
                            qcv[:, :, 0:9], xp3[:, :, c, :], wqp_l[c])
                        nc.vector.tensor_scalar_mul(
                            qcv[:, :, 9:18], xp3[:, :, c, :], wkp_l[c])
                    else:
                        nc.vector.scalar_tensor_tensor(
                            qcv[:, :, 0:9], xp3[:, :, c, :], wqp_l[c],
                            qcv[:, :, 0:9], op0=ALU.mult, op1=ALU.add)
                        nc.vector.scalar_tensor_tensor(
                            qcv[:, :, 9:18], xp3[:, :, c, :], wkp_l[c],
                            qcv[:, :, 9:18], op0=ALU.mult, op1=ALU.add)

                for j in range(NBLK):
                    gb0 = j * GB  # g offset within chunk
                    # views for this block (8 g)
                    xpb = xpc[:, gb0 * 63:(gb0 + GB) * 63].rearrange(
                        "p (g c n) -> p g c n", c=7, n=9)
                    xb = xc[:, gb0 * 9:(gb0 + GB) * 9].rearrange(
                        "p (g n) -> p g n", n=9)
                    qb = Qc[:, gb0 * 18:(gb0 + GB) * 18].rearrange(
                        "p (g t) -> p g t", t=18)

                    # --- energies (no pad; exp writes into T2/TI slot layouts) ---
                    E = pa.tile([128, GB * 81], bf16, tag="E")
                    E4 = E.rearrange("p (g i n) -> p g i n", i=9, n=9)
                    qpA = qb[:, :, 0:9].unsqueeze(3).broadcast_to((128, GB, 9, 9))
                    kpA = qb[:, :, 9:18].unsqueeze(2).broadcast_to((128, GB, 9, 9))
                    nc.gpsimd.tensor_mul(E4[:], qpA, kpA)
                    EI = pa.tile([128, GB * 81], bf16, tag="EI")
                    EI4 = EI.rearrange("p (g i n) -> p g i n", i=9, n=9)
                    xiA = xb.unsqueeze(3).broadcast_to((128, GB, 9, 9))
                    xnA = xb.unsqueeze(2).broadcast_to((128, GB, 9, 9))
                    nc.gpsimd.tensor_mul(EI4[:], xiA, xnA)

                    # --- T2: slots 0-6 = xp_c * E2, slot 7 = E2 (for D) ---
                    T2 = pa.tile([128, GB * 720], bf16, tag="T2")
                    T25 = T2.rearrange("p (g c i n) -> p g c i n", c=8, i=9, n=10)
                    nc.gpsimd.memset(T25[:, :, 7, :, 9], 0.0)
                    nc.scalar.activation(T25[:, :, 7, :, 0:9], E4[:], AF.Exp)
                    E24 = T25[:, :, 7, :, :]  # [p, g, 9i, 10n]
                    xpb_u = xpb.unsqueeze(3)
                    ap10 = [list(p) for p in xpb_u.ap]
                    ap10[-1][1] = 10  # read 10 consecutive (1 slack elem, x0)
                    xpbA = bass.AP(xpb_u.tensor, xpb_u.offset, ap10).broadcast_to(
                        (128, GB, 7, 9, 10))
                    e2A = E24.unsqueeze(2).broadcast_to((128, GB, 7, 9, 10))
                    nc.vector.tensor_mul(T25[:, :, 0:7, :, :], xpbA, e2A)

                    # --- TI: slot 0 = x_n * EI2, slot 1 = EI2 (for D_img) ---
                    TI = pa.tile([128, GB * 180], bf16, tag="TI")
                    TI5 = TI.rearrange("p (g c i n) -> p g c i n", c=2, i=9, n=10)
                    nc.gpsimd.memset(TI5[:, :, 1, :, 9], 0.0)
                    nc.scalar.activation(TI5[:, :, 1, :, 0:9], EI4[:], AF.Exp,
                                         scale=c0)
                    xb_u = xb.unsqueeze(2)
                    xap10 = [list(p) for p in xb_u.ap]
                    xap10[-1][1] = 10
                    xbA = bass.AP(xb_u.tensor, xb_u.offset, xap10).broadcast_to(
                        (128, GB, 9, 10))
                    nc.gpsimd.tensor_mul(TI5[:, :, 0, :, :], xbA,
                                         TI5[:, :, 1, :, :])

                    # --- tree-add reductions over n (replaces TensorReduce) ---
                    L1 = pa.tile([128, GB * 360], bf16, tag="L1")
                    L1v = L1.rearrange("p (g c i n) -> p g c i n", c=8, i=9, n=5)
                    nc.vector.tensor_tensor(
                        L1v[:], T25[:, :, :, :, 0:5], T25[:, :, :, :, 5:10],
                        op=ALU.add)
                    L2 = pa.tile([128, GB * 144], bf16, tag="L2")
                    L2v = L2.rearrange("p (g c i n) -> p g c i n", c=8, i=9, n=2)
                    nc.vector.tensor_tensor(
                        L2v[:], L1v[:, :, :, :, 0:2], L1v[:, :, :, :, 2:4],
                        op=ALU.add)
                    GT = pa.tile([128, GB * 72], f32, tag="GT")
                    GTv = GT.rearrange("p (g c i) -> p g c i", c=8, i=9)
                    nc.vector.tensor_tensor(
                        GTv[:], L2v[:, :, :, :, 0], L2v[:, :, :, :, 1], op=ALU.add)
                    nc.vector.tensor_tensor(
                        GTv[:], GTv[:], L1v[:, :, :, :, 4], op=ALU.add)

                    M1i = pa.tile([128, GB * 90], bf16, tag="M1i")
                    M1v = M1i.rearrange("p (g c i n) -> p g c i n", c=2, i=9, n=5)
                    nc.gpsimd.tensor_tensor(
                        M1v[:], TI5[:, :, :, :, 0:5], TI5[:, :, :, :, 5:10],
                        op=ALU.add)
                    M2 = pa.tile([128, GB * 36], bf16, tag="M2i")
                    M2v = M2.rearrange("p (g c i n) -> p g c i n", c=2, i=9, n=2)
                    nc.gpsimd.tensor_tensor(
                        M2v[:], M1v[:, :, :, :, 0:2], M1v[:, :, :, :, 2:4],
                        op=ALU.add)
                    GI2 = pa.tile([128, GB * 18], f32, tag="GI2")
                    GIv = GI2.rearrange("p (g c i) -> p g c i", c=2, i=9)
                    nc.gpsimd.tensor_tensor(
                        GIv[:], M2v[:, :, :, :, 0], M2v[:, :, :, :, 1], op=ALU.add)
                    nc.gpsimd.tensor_tensor(
                        GIv[:], GIv[:], M1v[:, :, :, :, 4], op=ALU.add)

                    # --- reciprocals of the two denominators ---
                    R = pa.tile([128, GB * 18], f32, tag="R")
                    Rv = R.rearrange("p (g t) -> p g t", t=18)
                    nc.vector.reciprocal(Rv[:, :, 0:9], GTv[:, :, 7, :])
                    nc.vector.reciprocal(Rv[:, :, 9:18], GIv[:, :, 1, :])

                    # --- attention outputs, laid out (g, c*9+i) for transpose ---
                    OCt = pa.tile([128, GB * 72], bf16, tag="OC")
                    OCv = OCt.rearrange("p (g c i) -> p g c i", c=8, i=9)
                    rpA = Rv[:, :, 0:9].unsqueeze(2).broadcast_to((128, GB, 7, 9))
                    nc.gpsimd.tensor_mul(OCv[:, :, 0:7, :], GTv[:, :, 0:7, :], rpA)
                    nc.gpsimd.tensor_mul(OCv[:, :, 7, :], GIv[:, :, 0, :],
                                         Rv[:, :, 9:18])

                    # ---- PE stage: 2 groups of 4 g ----
                    for h2 in range(NGRP):
                        ggl = [h2 * GG + t for t in range(GG)]
                        ps_tr = pps.tile([128, 512], bf16, tag="trans")
                        for t, gg in enumerate(ggl):
                            nc.tensor.transpose(
                                ps_tr[0:72, t * 128:(t + 1) * 128],
                                OCt[:, gg * 72:(gg + 1) * 72],
                                ident)
                        oct = pm.tile([128, 512], bf16, tag="oct")
                        nc.vector.tensor_copy(oct[0:72, :], ps_tr[0:72, :])
                        octr = oct

                        y2cat = py2.tile([128, 3072], fp8, tag="y2cat")
                        y2v = y2cat.rearrange("p (s n) -> p s n", s=6)
                        if gidx[0] < 2:
                            # zero the never-written fp8 slots once per buffer
                            nc.gpsimd.memset(y2cat[64:128, 2048:2560], 0.0)
                            nc.gpsimd.memset(y2cat[:, 2560:3072], 0.0)
                        gidx[0] += 1

                        def evac(eng, dst, src, bias, rows=128):
                            # psum f32 -> sbuf relu(x + bias), engine-selectable
                            if eng == 'A':
                                nc.scalar.activation(dst, src, AF.Relu, bias=bias)
                            elif eng == 'P':
                                nc.gpsimd.tensor_scalar(
                                    dst, src, bias, 0.0, op0=ALU.add, op1=ALU.max)
                            else:
                                nc.vector.tensor_scalar(
                                    dst, src, bias, 0.0, op0=ALU.add, op1=ALU.max)

                        Y1E = ['A', 'D', 'A', 'A', 'D']
                        Y2E = ['A', 'D', 'A', 'D', 'A']
                        for t in range(5):
                            ps1 = pps.tile([128, 512], f32, tag="y1")
                            if t < 4:
                                nc.tensor.matmul(
                                    ps1[:], wslice(f'w1quad{t}'),
                                    octr[0:72, :],
                                    start=True, stop=True)
                                y1sb = pm.tile([128, 512], bf16, tag="y1sb")
                                evac(Y1E[t], y1sb[:], ps1[:], cb_t[:, 0:1])
                                ps2 = pps.tile([128, 512], f32, tag="y2")
                                nc.tensor.matmul(
                                    ps2[:], wslice('w2pair'),
                                    y1sb[:],
                                    start=True, stop=True)
                                evac(Y2E[t], y2cat[:, t * 512:(t + 1) * 512],
                                     ps2[:], cb_t[:, 1:2])
                            else:
                                nc.tensor.matmul(
                                    ps1[0:64, :], wslice('w1s'),
                                    octr[0:72, :], start=True, stop=True)
                                y1sb = pm.tile([128, 512], bf16, tag="y1sb")
                                evac(Y1E[t], y1sb[0:64, :], ps1[0:64, :],
                                     cb_t[0:64, 5:6])
                                ps2 = pps.tile([128, 512], f32, tag="y2")
                                nc.tensor.matmul(
                                    ps2[0:64, :], wslice('w2s'),
                                    y1sb[0:64, :],
                                    start=True, stop=True)
                                evac(Y2E[t], y2cat[0:64, 2048:2560],
                                     ps2[0:64, :], cb_t[0:64, 6:7])

                        h8 = pm.tile([128, 1024], fp8, tag="h8")
                        h8v = h8.rearrange("p (s n) -> p s n", s=2)
                        for ch in range(2):
                            psh = pps.tile([128, 512], f32, tag="h")
                            for p3 in range(3):
                                nc.tensor.matmul(
                                    psh[:], w8pair(f'fc1_{ch}_{p3}'),
                                    y2v[:, 2 * p3:2 * p3 + 2, :],
                                    start=(p3 == 0), stop=(p3 == 2),
                                    perf_mode=DR)
                            evac('A' if ch == 0 else 'D',
                                 h8[:, ch * 512:(ch + 1) * 512], psh[:],
                                 cb_t[:, 2 + ch:3 + ch])

                        psy32 = pps.tile([32, 512], f32, tag="y2")
                        nc.tensor.matmul(psy32[:], w8pair('fw2dr'), h8v,
                                         start=True, stop=True, perf_mode=DR)
                        psy = psy32[0:2, :]

                        # sigmoid(x) = 0.5*tanh(0.5*x) + 0.5  (keeps exp table set)
                        if j == 0 and h2 == 0:
                            Ysb = pys.tile([2, GC * 128], f32, tag="Y")
                        gl0 = j * GB + h2 * GG  # g offset within chunk
                        yview = Ysb.rearrange("c (b g) -> c g b", g=GC)
                        src = psy.rearrange("c (g b) -> c g b", g=GG)
                        nc.scalar.activation(
                            yview[:, gl0:gl0 + GG, :], src, AF.Tanh, scale=0.5,
                            bias=cb_t[0:2, 4:5])

                # ---- output DMA for the chunk ----
                ydst = yv[:, :, k * GC:(k + 1) * GC]  # [2, 128, GC]
                ysrc = Ysb.rearrange("c (b g) -> c b g", g=GC)
                nc.sync.dma_start(ydst, ysrc)

    nc.compile()

    in_maps = []
    for core in range(NC):
        sl = slice(core * Bc, (core + 1) * Bc)
        xpt = np.ascontiguousarray(
            xp[sl].reshape(128, Bc // 128, 63).transpose(2, 1, 0)
        ).astype(ml_dtypes.bfloat16)
        in_maps.append({
            "xin": np.ascontiguousarray(x[sl].reshape(-1)),
            "xpin": np.ascontiguousarray(xp[sl].reshape(-1)),
            "xptin": xpt.reshape(63, Bc),
            "cw": cw_np_bf,
            "cw8": cw8_np,
            "cb": cb_np,
        })
    return nc, in_maps


def kernel(**inputs):
    from concourse.bass_utils import run_bass_kernel_spmd
    nc, in_maps = _build(inputs)
    kernel._last_nc = nc
    res = run_bass_kernel_spmd(nc, in_maps, core_ids=list(range(NC)))
    kernel._last_result = res
    Bc = in_maps[0]["xin"].size // 9
    outs = []
    for core in range(NC):
        yc = res.results[core]["yout"]          # [2, 128, G] = (cls, p, g)
        outs.append(yc.transpose(1, 2, 0).reshape(-1) * 0.5 + 0.5)  # rows r=p*G+g
    return np.concatenate(outs)



# revision 31
# speedup vs baseline: 1.5620x; 1.0265x over previous
import numpy as np

EPS = 1e-5
NC = 8
H = W = 3
N = 9
OC = 32
CP = 7
HN = 256
P = 128  # partitions


def _fold_consts(inp):
    """Host-side folding of all weights into matmul-ready constants."""
    f = lambda k: np.asarray(inp[k], np.float32)
    # image branch
    s1 = f('g1') / np.sqrt(f('v1') + EPS)
    A1 = f('w1')[:, 0] * f('wv')[0, 0] * s1 if 'wv' in inp else None
    return None


def _build(inputs):
    import concourse.bass as bass
    import concourse.bacc as bacc
    import concourse.tile as tile
    from concourse import mybir
    from concourse.bass_utils import run_bass_kernel_spmd

    dt = mybir.dt
    AF = mybir.ActivationFunctionType
    ALU = mybir.AluOpType

    x = np.asarray(inputs['x'], np.float32)
    xp = np.asarray(inputs['x_param'], np.float32)
    B = x.shape[0]
    Bc = B // NC            # rows per core
    G = Bc // P             # 128 g-groups per partition

    g = lambda k: np.asarray(inputs[k], np.float32)

    # ---------------- host-side constant folding ----------------
    # image branch (CIN=1, IC=1)
    wq, wk, wv = g('wq'), g('wk'), g('wv')
    c0 = float(wq[0, 0] * wk[0, 0])           # energy scale for image branch
    s1 = g('g1') / np.sqrt(g('v1') + EPS)
    A1 = g('w1')[:, 0] * wv[0, 0] * s1        # [32]
    C1 = (g('b1') - g('m1')) * s1 + g('be1')
    s2 = g('g2') / np.sqrt(g('v2') + EPS)
    W2i = g('w2') * s2[:, None]               # [32,32] row-scaled
    C2i = (g('b2') - g('m2')) * s2 + g('be2')
    # param branch
    wqp, wkp, wvp = g('wqp'), g('wkp'), g('wvp')
    s1p = g('g1p') / np.sqrt(g('v1p') + EPS)
    W1v = (g('w1p') * s1p[:, None]) @ wvp     # [32,7]
    C1p = (g('b1p') - g('m1p')) * s1p + g('be1p')
    s2p = g('g2p') / np.sqrt(g('v2p') + EPS)
    W2p = g('w2p') * s2p[:, None]
    C2p = (g('b2p') - g('m2p')) * s2p + g('be2p')
    fw1, fb1, fw2, fb2 = g('fw1'), g('fb1'), g('fw2'), g('fb2')

    # y1cat feature order: j<32 param branch, j>=32 image branch
    # oc rows are ordered r = c*9 + i (c: 0-6 param chans, 7 img; i: position)
    def w1block(positions):
        # lhsT [72, len(positions)*64] for the given output positions
        m = np.zeros((72, len(positions) * 64), np.float32)
        for ii, i in enumerate(positions):
            for c in range(7):
                m[c * 9 + i, ii * 64:ii * 64 + 32] = W1v[:, c]
            m[7 * 9 + i, ii * 64 + 32:ii * 64 + 64] = A1
        return m
    b1all = np.concatenate([C1p, C1]).astype(np.float32)          # [64]
    b1pair = np.concatenate([b1all, b1all]).astype(np.float32)    # [128]

    W2s = np.zeros((64, 64), np.float32)      # lhsT[k=y1feat, m=y2feat]
    W2s[0:32, 0:32] = W2p.T
    W2s[32:64, 32:64] = W2i.T
    b2all = np.concatenate([C2p, C2i]).astype(np.float32)
    W2pair = np.zeros((128, 128), np.float32)
    W2pair[0:64, 0:64] = W2s
    W2pair[64:128, 64:128] = W2s
    b2pair = np.concatenate([b2all, b2all]).astype(np.float32)

    # fc1 weight rearranged per position i: rows = (i_local, j), cols = hn
    def catidx(j, i):
        if j < 32:
            return 288 + j * 9 + i            # param block of cat
        return (j - 32) * 9 + i               # image block of cat

    # fc1 weights as fp8 DoubleRow k-subtile pairs.
    # y2cat subtile S holds: S<4 -> rows (ii*64+j) = feature (j, i=2S+ii);
    # S=4 -> rows j<64 = feature (j, i=8), rows 64:128 zero; S=5 -> zero.
    def m1row(S, k):
        if S < 4:
            ii, j = k // 64, k % 64
            return catidx(j, 2 * S + ii)
        if S == 4 and k < 64:
            return catidx(k, 8)
        return None
    M1DR = np.zeros((2, 3, 128, 2, 128), np.float32)  # [half, pair, k, s, m]
    for h in range(2):
        for p3 in range(3):
            for s in range(2):
                S = 2 * p3 + s
                for k in range(128):
                    ci = m1row(S, k)
                    if ci is not None:
                        M1DR[h, p3, k, s, :] = fw1[h * 128:(h + 1) * 128, ci]

    fw2T = fw2.T.astype(np.float32)           # [256, 2]
    FW2DR = np.zeros((128, 2, 32), np.float32)  # M=32: dual-fp8 ldweights needs M>=32
    FW2DR[:, 0, 0:2] = fw2T[0:128]
    FW2DR[:, 1, 0:2] = fw2T[128:256]

    # ---- pack weight constants into one [128, Fw] tensor ----
    cols = {}
    off = 0
    def put(name, arr, row0=0):
        nonlocal off
        a = np.zeros((128, arr.shape[1]), np.float32)
        a[row0:row0 + arr.shape[0]] = arr
        cols[name] = (off, arr.shape[1], row0 + arr.shape[0], row0)
        off += arr.shape[1]
        return a
    blocks = []
    blocks.append(put('id', np.eye(128, dtype=np.float32)))
    blocks.append(put('w2pair', W2pair))
    for t in range(4):
        blocks.append(put(f'w1quad{t}', w1block([2 * t, 2 * t + 1])))
    blocks.append(put('w1s', w1block([8])))
    blocks.append(put('w2s', W2s))
    # q/k weights for the PE-side flipped matmul: lhsT = xpT slice [63, 128],
    # rhs = Wqk [63, 18]; Wqk[c*9+n, i] = wqp[c]*(n==i) (cols 0-8) / wkp (9-17)
    Wqk = np.zeros((63, 18), np.float32)
    for c in range(7):
        for i in range(9):
            Wqk[c * 9 + i, i] = wqp[0, c]
            Wqk[c * 9 + i, 9 + i] = wkp[0, c]
    blocks.append(put('wqk', Wqk))
    cw_np = np.concatenate(blocks, axis=1)
    import ml_dtypes
    cw_np_bf = cw_np.astype(ml_dtypes.bfloat16)
    Fw = cw_np.shape[1]

    # ---- fp8 weight constants (fc1 DR pairs + fc2 DR) ----
    cols8 = {}
    off8 = 0
    blocks8 = []
    def put8(name, arr):  # arr [128, w]
        nonlocal off8
        cols8[name] = (off8, arr.shape[1])
        off8 += arr.shape[1]
        blocks8.append(arr.astype(np.float32))
    for h in range(2):
        for p3 in range(3):
            put8(f'fc1_{h}_{p3}', M1DR[h, p3].reshape(128, 256))
    put8('fw2dr', FW2DR.reshape(128, 64))
    cw8_np = np.concatenate(blocks8, axis=1).astype(ml_dtypes.float8_e4m3)
    F8 = cw8_np.shape[1]

    cb_np = np.zeros((128, 8), np.float32)
    cb_np[:, 0] = b1pair
    cb_np[:, 1] = b2pair
    cb_np[:, 2] = fb1[0:128]
    cb_np[:, 3] = fb1[128:256]
    cb_np[0:2, 4] = fb2 * 0.5
    cb_np[0:64, 5] = b1all
    cb_np[0:64, 6] = b2all

    # ---------------- build the bass program ----------------
    nc = bacc.Bacc("TRN2", target_bir_lowering=False, debug=False)
    f32, f32r, bf16 = dt.float32, dt.float32r, dt.bfloat16
    fp8 = dt.float8e4
    DR = mybir.MatmulPerfMode.DoubleRow

    x_d = nc.dram_tensor("xin", [Bc * 9], f32, kind="ExternalInput").ap()
    xp_d = nc.dram_tensor("xpin", [Bc * 63], f32, kind="ExternalInput").ap()
    xpt_d = nc.dram_tensor("xptin", [63, Bc], bf16, kind="ExternalInput").ap()
    cw_d = nc.dram_tensor("cw", [128, Fw], bf16, kind="ExternalInput").ap()
    cw8_d = nc.dram_tensor("cw8", [128, F8], fp8, kind="ExternalInput").ap()
    cb_d = nc.dram_tensor("cb", [128, 8], f32, kind="ExternalInput").ap()
    y_d = nc.dram_tensor("yout", [2, P, G], f32, kind="ExternalOutput").ap()

    xv = x_d.rearrange("(p f) -> p f", p=P)     # [128, G*9]
    xpv = xp_d.rearrange("(p f) -> p f", p=P)   # [128, G*63]
    yv = y_d                                     # [2, 128, 128]

    NCH = 4                  # dma chunks over g
    GC = G // NCH            # 32 g per chunk
    NBLK = 4                 # blocks per chunk (8 g each)
    GB = GC // NBLK          # 8 g per block
    NGRP = 2                 # groups per block (4 g each)
    GG = GB // NGRP          # 4

    wqp_l = [float(v) for v in wqp[0]]
    wkp_l = [float(v) for v in wkp[0]]

    with tile.TileContext(nc) as tc:
        with (
            tc.tile_pool(name="consts", bufs=1) as pc,
            tc.tile_pool(name="pin", bufs=2) as pin,
            tc.tile_pool(name="pq", bufs=2) as pq,
            tc.tile_pool(name="patt", bufs=3) as pa,
            tc.tile_pool(name="pmm", bufs=3) as pm,
            tc.tile_pool(name="py2", bufs=2) as py2,
            tc.tile_pool(name="pys", bufs=2) as pys,
            tc.tile_pool(name="pps", bufs=2, space="PSUM") as pps,
        ):
            cw_t = pc.tile([128, Fw], bf16)
            nc.sync.dma_start(cw_t[:], cw_d)
            cw8_t = pc.tile([128, F8], fp8)
            nc.sync.dma_start(cw8_t[:], cw8_d)
            cb_t = pc.tile([128, 8], f32)
            nc.sync.dma_start(cb_t[:], cb_d)

            def wslice(name):
                o, w_, r1, r0 = cols[name]
                return cw_t[r0:r1, o:o + w_]

            def w8pair(name):
                o, w_ = cols8[name]
                return cw8_t[:, o:o + w_].rearrange("p (s m) -> p s m", s=2)

            ident = wslice('id')
            gidx = [0]

            for k in range(NCH):
                # ---- input DMA for this chunk ----
                xpc = pin.tile([128, GC * 63 + 16], bf16, tag="xp")
                nc.gpsimd.memset(xpc[:, GC * 63:], 0.0)
                nc.gpsimd.dma_start(xpc[:, 0:GC * 63], xpv[:, k * GC * 63:(k + 1) * GC * 63])
                xc = pin.tile([128, GC * 9 + 16], bf16, tag="x")
                nc.gpsimd.memset(xc[:, GC * 9:], 0.0)
                nc.gpsimd.dma_start(xc[:, 0:GC * 9],
                                    xv[:, k * GC * 9:(k + 1) * GC * 9])


                # ---- qp / kp for the whole chunk (32 g) ----
                Qc = pq.tile([128, GC * 18], bf16, tag="Q")
                qcv = Qc.rearrange("p (g t) -> p g t", t=18)
                xp3 = xpc[:, 0:GC * 63].rearrange("p (g c n) -> p g c n", c=7, n=9)
                for c in range(7):
                    if c == 0:
                        nc.vector.tensor_scalar_mul(
                            qcv[:, :, 0:9], xp3[:, :, c, :], wqp_l[c])
                        nc.vector.tensor_scalar_mul(
                            qcv[:, :, 9:18], xp3[:, :, c, :], wkp_l[c])
                    else:
                        nc.vector.scalar_tensor_tensor(
                            qcv[:, :, 0:9], xp3[:, :, c, :], wqp_l[c],
                            qcv[:, :, 0:9], op0=ALU.mult, op1=ALU.add)
                        nc.vector.scalar_tensor_tensor(
                            qcv[:, :, 9:18], xp3[:, :, c, :], wkp_l[c],
                            qcv[:, :, 9:18], op0=ALU.mult, op1=ALU.add)

                for j in range(NBLK):
                    gb0 = j * GB  # g offset within chunk
                    # views for this block (8 g)
                    xpb = xpc[:, gb0 * 63:(gb0 + GB) * 63].rearrange(
                        "p (g c n) -> p g c n", c=7, n=9)
                    xb = xc[:, gb0 * 9:(gb0 + GB) * 9].rearrange(
                        "p (g n) -> p g n", n=9)
                    qb = Qc[:, gb0 * 18:(gb0 + GB) * 18].rearrange(
                        "p (g t) -> p g t", t=18)

                    # --- energies (no pad; exp writes into T2/TI slot layouts) ---
                    E = pa.tile([128, GB * 81], bf16, tag="E")
                    E4 = E.rearrange("p (g i n) -> p g i n", i=9, n=9)
                    qpA = qb[:, :, 0:9].unsqueeze(3).broadcast_to((128, GB, 9, 9))
                    kpA = qb[:, :, 9:18].unsqueeze(2).broadcast_to((128, GB, 9, 9))
                    nc.gpsimd.tensor_mul(E4[:], qpA, kpA)
                    EI = pa.tile([128, GB * 81], bf16, tag="EI")
                    EI4 = EI.rearrange("p (g i n) -> p g i n", i=9, n=9)
                    xiA = xb.unsqueeze(3).broadcast_to((128, GB, 9, 9))
                    xnA = xb.unsqueeze(2).broadcast_to((128, GB, 9, 9))
                    nc.gpsimd.tensor_mul(EI4[:], xiA, xnA)

                    # --- T2: slots 0-6 = xp_c * E2, slot 7 = E2 (for D) ---
                    T2 = pa.tile([128, GB * 720], bf16, tag="T2")
                    T25 = T2.rearrange("p (g c i n) -> p g c i n", c=8, i=9, n=10)
                    nc.gpsimd.memset(T25[:, :, 7, :, 9], 0.0)
                    nc.scalar.activation(T25[:, :, 7, :, 0:9], E4[:], AF.Exp)
                    E24 = T25[:, :, 7, :, :]  # [p, g, 9i, 10n]
                    xpb_u = xpb.unsqueeze(3)
                    ap10 = [list(p) for p in xpb_u.ap]
                    ap10[-1][1] = 10  # read 10 consecutive (1 slack elem, x0)
                    xpbA = bass.AP(xpb_u.tensor, xpb_u.offset, ap10).broadcast_to(
                        (128, GB, 7, 9, 10))
                    e2A = E24.unsqueeze(2).broadcast_to((128, GB, 7, 9, 10))
                    nc.vector.tensor_mul(T25[:, :, 0:7, :, :], xpbA, e2A)

                    # --- TI: slot 0 = x_n * EI2, slot 1 = EI2 (for D_img) ---
                    TI = pa.tile([128, GB * 180], bf16, tag="TI")
                    TI5 = TI.rearrange("p (g c i n) -> p g c i n", c=2, i=9, n=10)
                    nc.gpsimd.memset(TI5[:, :, 1, :, 9], 0.0)
                    nc.scalar.activation(TI5[:, :, 1, :, 0:9], EI4[:], AF.Exp,
                                         scale=c0)
                    xb_u = xb.unsqueeze(2)
                    xap10 = [list(p) for p in xb_u.ap]
                    xap10[-1][1] = 10
                    xbA = bass.AP(xb_u.tensor, xb_u.offset, xap10).broadcast_to(
                        (128, GB, 9, 10))
                    nc.gpsimd.tensor_mul(TI5[:, :, 0, :, :], xbA,
                                         TI5[:, :, 1, :, :])

                    # --- tree-add reductions over n (replaces TensorReduce) ---
                    L1 = pa.tile([128, GB * 360], bf16, tag="L1")
                    L1v = L1.rearrange("p (g c i n) -> p g c i n", c=8, i=9, n=5)
                    nc.vector.tensor_tensor(
                        L1v[:], T25[:, :, :, :, 0:5], T25[:, :, :, :, 5:10],
                        op=ALU.add)
                    L2 = pa.tile([128, GB * 144], bf16, tag="L2")
                    L2v = L2.rearrange("p (g c i n) -> p g c i n", c=8, i=9, n=2)
                    nc.vector.tensor_tensor(
                        L2v[:], L1v[:, :, :, :, 0:2], L1v[:, :, :, :, 2:4],
                        op=ALU.add)
                    GT = pa.tile([128, GB * 72], f32, tag="GT")
                    GTv = GT.rearrange("p (g c i) -> p g c i", c=8, i=9)
                    nc.vector.tensor_tensor(
                        GTv[:], L2v[:, :, :, :, 0], L2v[:, :, :, :, 1], op=ALU.add)
                    nc.vector.tensor_tensor(
                        GTv[:], GTv[:], L1v[:, :, :, :, 4], op=ALU.add)

                    M1i = pa.tile([128, GB * 90], bf16, tag="M1i")
                    M1v = M1i.rearrange("p (g c i n) -> p g c i n", c=2, i=9, n=5)
                    nc.gpsimd.tensor_tensor(
                        M1v[:], TI5[:, :, :, :, 0:5], TI5[:, :, :, :, 5:10],
                        op=ALU.add)
                    M2 = pa.tile([128, GB * 36], bf16, tag="M2i")
                    M2v = M2.rearrange("p (g c i n) -> p g c i n", c=2, i=9, n=2)
                    nc.gpsimd.tensor_tensor(
                        M2v[:], M1v[:, :, :, :, 0:2], M1v[:, :, :, :, 2:4],
                        op=ALU.add)
                    GI2 = pa.tile([128, GB * 18], f32, tag="GI2")
                    GIv = GI2.rearrange("p (g c i) -> p g c i", c=2, i=9)
                    nc.gpsimd.tensor_tensor(
                        GIv[:], M2v[:, :, :, :, 0], M2v[:, :, :, :, 1], op=ALU.add)
                    nc.gpsimd.tensor_tensor(
                        GIv[:], GIv[:], M1v[:, :, :, :, 4], op=ALU.add)

                    # --- reciprocals of the two denominators ---
                    R = pa.tile([128, GB * 18], f32, tag="R")
                    Rv = R.rearrange("p (g t) -> p g t", t=18)
                    nc.vector.reciprocal(Rv[:, :, 0:9], GTv[:, :, 7, :])
                    nc.vector.reciprocal(Rv[:, :, 9:18], GIv[:, :, 1, :])

                    # --- attention outputs, laid out (g, c*9+i) for transpose ---
                    OCt = pa.tile([128, GB * 72], bf16, tag="OC")
                    OCv = OCt.rearrange("p (g c i) -> p g c i", c=8, i=9)
                    rpA = Rv[:, :, 0:9].unsqueeze(2).broadcast_to((128, GB, 7, 9))
                    nc.gpsimd.tensor_mul(OCv[:, :, 0:7, :], GTv[:, :, 0:7, :], rpA)
                    nc.gpsimd.tensor_mul(OCv[:, :, 7, :], GIv[:, :, 0, :],
                                         Rv[:, :, 9:18])

                    # ---- PE stage: 2 groups of 4 g ----
                    for h2 in range(NGRP):
                        ggl = [h2 * GG + t for t in range(GG)]
                        ps_tr = pps.tile([128, 512], bf16, tag="trans")
                        for t, gg in enumerate(ggl):
                            nc.tensor.transpose(
                                ps_tr[0:72, t * 128:(t + 1) * 128],
                                OCt[:, gg * 72:(gg + 1) * 72],
                                ident)
                        oct = pm.tile([128, 512], bf16, tag="oct")
                        nc.vector.tensor_copy(oct[0:72, :], ps_tr[0:72, :])
                        octr = oct

                        y2cat = py2.tile([128, 3072], fp8, tag="y2cat")
                        y2v = y2cat.rearrange("p (s n) -> p s n", s=6)
                        if gidx[0] < 2:
                            # zero the never-written fp8 slots once per buffer
                            nc.gpsimd.memset(y2cat[64:128, 2048:2560], 0.0)
                            nc.gpsimd.memset(y2cat[:, 2560:3072], 0.0)
                        gidx[0] += 1

                        def evac(eng, dst, src, bias, rows=128):
                            # psum f32 -> sbuf relu(x + bias), engine-selectable
                            if eng == 'A':
                                nc.scalar.activation(dst, src, AF.Relu, bias=bias)
                            elif eng == 'P':
                                nc.gpsimd.tensor_scalar(
                                    dst, src, bias, 0.0, op0=ALU.add, op1=ALU.max)
                            else:
                                nc.vector.tensor_scalar(
                                    dst, src, bias, 0.0, op0=ALU.add, op1=ALU.max)

                        Y1E = ['A', 'D', 'A', 'A', 'A']
                        Y2E = ['A', 'D', 'A', 'D', 'A']
                        for t in range(5):
                            ps1 = pps.tile([128, 512], f32, tag="y1")
                            if t < 4:
                                nc.tensor.matmul(
                                    ps1[:], wslice(f'w1quad{t}'),
                                    octr[0:72, :],
                                    start=True, stop=True)
                                y1sb = pm.tile([128, 512], bf16, tag="y1sb")
                                evac(Y1E[t], y1sb[:], ps1[:], cb_t[:, 0:1])
                                ps2 = pps.tile([128, 512], f32, tag="y2")
                                nc.tensor.matmul(
                                    ps2[:], wslice('w2pair'),
                                    y1sb[:],
                                    start=True, stop=True)
                                evac(Y2E[t], y2cat[:, t * 512:(t + 1) * 512],
                                     ps2[:], cb_t[:, 1:2])
                            else:
                                nc.tensor.matmul(
                                    ps1[0:64, :], wslice('w1s'),
                                    octr[0:72, :], start=True, stop=True)
                                y1sb = pm.tile([128, 512], bf16, tag="y1sb")
                                evac(Y1E[t], y1sb[0:64, :], ps1[0:64, :],
                                     cb_t[0:64, 5:6])
                                ps2 = pps.tile([128, 512], f32, tag="y2")
                                nc.tensor.matmul(
                                    ps2[0:64, :], wslice('w2s'),
                                    y1sb[0:64, :],
                                    start=True, stop=True)
                                evac(Y2E[t], y2cat[0:64, 2048:2560],
                                     ps2[0:64, :], cb_t[0:64, 6:7])

                        h8 = pm.tile([128, 1024], fp8, tag="h8")
                        h8v = h8.rearrange("p (s n) -> p s n", s=2)
                        for ch in range(2):
                            psh = pps.tile([128, 512], f32, tag="h")
                            for p3 in range(3):
                                nc.tensor.matmul(
                                    psh[:], w8pair(f'fc1_{ch}_{p3}'),
                                    y2v[:, 2 * p3:2 * p3 + 2, :],
                                    start=(p3 == 0), stop=(p3 == 2),
                                    perf_mode=DR)
                            evac('A' if ch == 0 else 'D',
                                 h8[:, ch * 512:(ch + 1) * 512], psh[:],
                                 cb_t[:, 2 + ch:3 + ch])

                        psy32 = pps.tile([32, 512], f32, tag="y2")
                        nc.tensor.matmul(psy32[:], w8pair('fw2dr'), h8v,
                                         start=True, stop=True, perf_mode=DR)
                        psy = psy32[0:2, :]

                        # sigmoid(x) = 0.5*tanh(0.5*x) + 0.5  (keeps exp table set)
                        if j == 0 and h2 == 0:
                            Ysb = pys.tile([2, GC * 128], f32, tag="Y")
                        gl0 = j * GB + h2 * GG  # g offset within chunk
                        yview = Ysb.rearrange("c (b g) -> c g b", g=GC)
                        src = psy.rearrange("c (g b) -> c g b", g=GG)
                        nc.scalar.activation(
                            yview[:, gl0:gl0 + GG, :], src, AF.Tanh, scale=0.5,
                            bias=cb_t[0:2, 4:5])

                # ---- output DMA for the chunk ----
                ydst = yv[:, :, k * GC:(k + 1) * GC]  # [2, 128, GC]
                ysrc = Ysb.rearrange("c (b g) -> c b g", g=GC)
                nc.sync.dma_start(ydst, ysrc)

    nc.compile()

    in_maps = []
    for core in range(NC):
        sl = slice(core * Bc, (core + 1) * Bc)
        xpt = np.ascontiguousarray(
            xp[sl].reshape(128, Bc // 128, 63).transpose(2, 1, 0)
        ).astype(ml_dtypes.bfloat16)
        in_maps.append({
            "xin": np.ascontiguousarray(x[sl].reshape(-1)),
            "xpin": np.ascontiguousarray(xp[sl].reshape(-1)),
            "xptin": xpt.reshape(63, Bc),
            "cw": cw_np_bf,
            "cw8": cw8_np,
            "cb": cb_np,
        })
    return nc, in_maps


def kernel(**inputs):
    from concourse.bass_utils import run_bass_kernel_spmd
    nc, in_maps = _build(inputs)
    kernel._last_nc = nc
    res = run_bass_kernel_spmd(nc, in_maps, core_ids=list(range(NC)))
    kernel._last_result = res
    Bc = in_maps[0]["xin"].size // 9
    outs = []
    for core in range(NC):
        yc = res.results[core]["yout"]          # [2, 128, G] = (cls, p, g)
        outs.append(yc.transpose(1, 2, 0).reshape(-1) * 0.5 + 0.5)  # rows r=p*G+g
    return np.concatenate(outs)

